# revision 11
# baseline (speedup 1.0000x reference)
"""Trainium2 Bass kernel for nn_HandGNNEncoder (2-layer GCN on 21-node hand
graphs + mean pool), data-parallel over 8 NeuronCores.

Math restructure (exact):
  reference: h1 = relu(A @ (x @ W1) + b1); out = mean_t(A @ (h1 @ W2) + b2)
  mean-pool is linear, so with m[s] = column-mean of A (all > 0):
      out[g] = sum_s m[s] * h1[g,s,:] @ W2 + b2
  m[s] > 0 folds inside the relu:  m*relu(z) = relu(m*z).
  Stage 1 (PE): z[(s,f), g] = TW.T @ x'[g]   with TW[(s',c),(s,f)] =
      m[s]*A[s,s']*W1[c,f], bias row via a constant-1 input row, plus one
      extra column that relu's to the constant 1 (carries b2 in stage 2).
  Stage 2 (PE): out[d, g] = sum_k W2R_k.T @ relu_k  accumulated in PSUM.

v3 performance structure (vs the chunk-major baseline):
  * PE warmup burst: ~20 dependency-free dummy matmuls issued while the
    input DMAs land. The PE HAM clock gate only un-throttles (1.2 GHz ->
    2.4 GHz) after a ~3.4us window of gap-free matmul activity, which the
    synced main loop never provides; the trace showed the whole baseline
    ran at 1.2 GHz.
  * Stage-1 row packing: K=43 uses only rows 0..42 of the 128x128 array.
    xt and TW are duplicated at partition offset 64 so chunk A streams on
    rows 0..42 while chunk B streams concurrently on rows 64..106
    (tile_position (0,0) / (64,0)) - stage-1 wall time halves.
  * k-inner over chunk pairs with [128,1024] PSUM tiles (A|B): one relu
    instruction covers both chunks (the per-instruction overhead on
    ACT/DVE is ~170-230 cycles, so FD=1024 is ~30% cheaper than 2x512),
    PSUM = 3x2 (stage-1) + 1x2 (out accum) = exactly 8 banks.
  * Input DMAs issued from 4 different engine queues in parallel; f16
    output halves the store traffic.
"""

import numpy as np

import concourse.bass as bass
import concourse.mybir as mybir
import concourse.tile as tile
from concourse import bass_utils

# ---- hardcoded problem constants ----
B, S, NNODE, CIN = 64, 512, 21, 2
D1, D2 = 64, 128
G = B * S                      # 32768 graphs
N_CORES = 8
G_CORE = G // N_CORES          # 4096 graphs per core
CHUNK = 512                    # graphs per matmul moving dim
PAIR = 2 * CHUNK               # graphs per pipeline pair
N_PAIRS = G_CORE // PAIR       # 4
K1 = NNODE * CIN + 1           # 43 contraction rows (42 feats + ones row)
KT = 11                        # 1408 / 128 k-tiles for stage 2
M1 = KT * 128                  # 1408 = 1344 (s,f) cols + 1 bias col + 63 pad
XROWS = 107                    # 43 rows + dup at partitions 64..106
SKEW = 3                       # stage-2 of k issued in period k+SKEW
N_WARM = 20                    # HAM warmup matmuls

EDGES = np.array(
    [[0, 1], [1, 2], [2, 3], [3, 4], [0, 5], [5, 6], [6, 7], [7, 8],
     [0, 9], [9, 10], [10, 11], [11, 12], [0, 13], [13, 14], [14, 15],
     [15, 16], [0, 17], [17, 18], [18, 19], [19, 20], [5, 9], [9, 13],
     [13, 17]], dtype=np.int64)


def fold_weights(W1, b1, W2, b2):
    """Fold adjacency, mean-pool and biases into two dense operands."""
    W1 = np.asarray(W1, np.float32)
    b1 = np.asarray(b1, np.float32)
    W2 = np.asarray(W2, np.float32)
    b2 = np.asarray(b2, np.float32)
    A = np.eye(NNODE, dtype=np.float32)
    A[EDGES[:, 1], EDGES[:, 0]] = 1.0
    deg = A.sum(axis=1)
    dis = 1.0 / np.sqrt(deg)
    a_norm = dis[:, None] * A * dis[None, :]          # [t, s] float32
    m = a_norm.mean(axis=0)                           # [21], all > 0

    # tw[(s',c), (s,f)] = m[s] * a_norm[s, s'] * W1[c, f]
    tw = np.zeros((K1, M1), np.float32)
    blk = np.einsum("s,st,cf->tcsf", m, a_norm, W1)   # [s'=t, c, s, f]
    tw[: NNODE * CIN, : NNODE * D1] = blk.reshape(NNODE * CIN, NNODE * D1)
    tw[K1 - 1, : NNODE * D1] = (m[:, None] * b1[None, :]).reshape(-1)
    tw[K1 - 1, NNODE * D1] = 1.0                      # relu's to constant 1

    w2full = np.zeros((M1, D2), np.float32)
    w2full[: NNODE * D1] = np.tile(W2, (NNODE, 1))
    w2full[NNODE * D1] = b2                           # rides the const-1 row
    # device tile is [128, KT*128] with pass-k slice [:, k*128:(k+1)*128]
    w2r = np.ascontiguousarray(
        w2full.reshape(KT, 128, D2).transpose(1, 0, 2).reshape(128, KT * D2))
    return tw, w2r


def build_bass(op_dt="float16"):
    f16 = getattr(mybir.dt, op_dt)
    f32 = mybir.dt.float32
    nc = bass.Bass("TRN2", target_bir_lowering=False, debug=False)
    xt_d = nc.dram_tensor("xt", [XROWS, G_CORE], f16, kind="ExternalInput").ap()
    tw_d = nc.dram_tensor("tw", [XROWS, M1], f16, kind="ExternalInput").ap()
    w2r_d = nc.dram_tensor("w2r", [128, KT * 128], f16,
                           kind="ExternalInput").ap()
    out_d = nc.dram_tensor("out", [D2, G_CORE], f16, kind="ExternalOutput").ap()

    relu = mybir.ActivationFunctionType.Relu
    copyf = mybir.ActivationFunctionType.Copy

    with tile.TileContext(nc) as tc:
        with (
            tc.tile_pool(name="w", bufs=1) as wpool,
            tc.tile_pool(name="rt", bufs=SKEW + 1) as rtpool,
            tc.tile_pool(name="ot", bufs=2) as otpool,
            tc.tile_pool(name="px", bufs=3, space="PSUM") as xpool,
            tc.tile_pool(name="po", bufs=1, space="PSUM") as opool,
        ):
            # --- tiles ---
            tw_t = wpool.tile([XROWS, M1], f16, tag="tw")
            xt_t = wpool.tile([XROWS, G_CORE], f16, tag="xt")
            w2r_t = wpool.tile([128, KT * 128], f16, tag="w2r")
            warm_t = wpool.tile([128, 640], f16, tag="warm")

            # --- input DMAs: 4 big transfers, 2 per HWDGE queue, ordered
            # by first use. Each dma_start costs ~2us end-to-end on its
            # queue (issue + transfer + sem), so fewer/bigger wins; all
            # land while the PE warmup burst runs. ---
            nc.vector.memset(warm_t, 0.0)
            nc.sync.dma_start(out=tw_t[0:XROWS], in_=tw_d[0:XROWS])
            nc.scalar.dma_start(out=xt_t[0:43], in_=xt_d[0:43])
            nc.sync.dma_start(out=xt_t[64:107], in_=xt_d[64:107])
            nc.scalar.dma_start(out=w2r_t[0:128], in_=w2r_d[0:128])

            # --- HAM warmup: dependency-free matmuls reusing one weight
            # load (ldweights=False on the repeats) -> gap-free PE stream
            # that un-throttles the clock gate (1.2 -> 2.4 GHz) before the
            # real pipeline starts. Writes land in a stage-1 PSUM buffer
            # and are cleared by the first real start=True matmul there. ---
            out_ps = opool.tile([D2, PAIR], f32, tag="po")
            warm_ps = xpool.tile([128, PAIR], f32, tag="px")
            for i in range(N_WARM):
                mm = nc.tensor.matmul(
                    warm_ps[:, 0:CHUNK],
                    lhsT=warm_t[:, 0:128],
                    rhs=warm_t[:, 128:128 + CHUNK],
                    start=True, stop=True,
                    skip_group_check=True,
                )
                if i > 0:
                    mm.ldweights = False

            tw_lo = [tw_t[0:43, k * 128:(k + 1) * 128] for k in range(KT)]
            tw_hi = [tw_t[64:107, k * 128:(k + 1) * 128] for k in range(KT)]
            w2r_sb = [w2r_t[:, k * 128:(k + 1) * 128] for k in range(KT)]

            ACT_KS = {0, 2, 4, 6, 8, 9, 10}   # relu k's on ScalarE; rest DVE

            for p in range(N_PAIRS):
                cs = p * PAIR
                rts = {}

                def mm3(k):
                    nc.tensor.matmul(
                        out_ps[:, 0:CHUNK],
                        lhsT=w2r_sb[k], rhs=rts[k][:, 0:CHUNK],
                        start=(k == 0), stop=(k == KT - 1),
                        skip_group_check=True,
                    )
                    nc.tensor.matmul(
                        out_ps[:, CHUNK:PAIR],
                        lhsT=w2r_sb[k], rhs=rts.pop(k)[:, CHUNK:PAIR],
                        start=(k == 0), stop=(k == KT - 1),
                        skip_group_check=True,
                    )

                for k in range(KT):
                    xk = xpool.tile([128, PAIR], f32, tag="px")
                    nc.tensor.matmul(
                        xk[:, 0:CHUNK],
                        lhsT=tw_lo[k], rhs=xt_t[0:43, cs:cs + CHUNK],
                        start=True, stop=True,
                        tile_position=(0, 0),
                        skip_group_check=True,
                    )
                    nc.tensor.matmul(
                        xk[:, CHUNK:PAIR],
                        lhsT=tw_hi[k],
                        rhs=xt_t[64:107, cs + CHUNK:cs + PAIR],
                        start=True, stop=True,
                        tile_position=(64, 0),
                        skip_group_check=True,
                    )
                    rt = rtpool.tile([128, PAIR], f16, tag="rt")
                    if k in ACT_KS:
                        nc.scalar.activation(out=rt, in_=xk, func=relu)
                    else:
                        nc.vector.tensor_scalar_max(out=rt, in0=xk,
                                                    scalar1=0.0)
                    rts[k] = rt
                    if k >= SKEW:
                        mm3(k - SKEW)
                for k in range(KT - SKEW, KT):
                    mm3(k)

                ot = otpool.tile([D2, PAIR], f16, tag="ot")
                nc.vector.tensor_copy(out=ot, in_=out_ps)
                nc.sync.dma_start(out=out_d[:, cs:cs + PAIR], in_=ot)
    _rebalance_matmul_waits(nc)
    return nc


def _rebalance_matmul_waits(nc):
    """Walrus' TPB ISA structs accept only one sync-wait per instruction on
    the compute engines, but Tile can attach several (PE completion-order +
    cross-engine WAR + DMA). Keep one wait on the instruction and move the
    excess onto the immediately-preceding Ldweights (for matmuls) or onto
    freshly inserted same-engine NoOps - those execute just before on the
    same in-order queue, so waiting there is the same or stronger ordering."""
    import bass_rust
    import concourse.mybir as mybir

    exempt = {"InstEventSemaphore", "InstUnconditionalBranch",
              "InstCall", "InstISA", "InstNoOp"}
    nop_ctr = [0]
    for fn in nc.m.functions:
        for blk in fn.blocks:
            insts = list(blk.instructions)
            out = []
            pending_free_ldw = None
            for inst in insts:
                tn = type(inst).__name__
                if tn == "InstLdweights":
                    si = inst.sync_info
                    nw = len(si.on_wait) if si is not None else 0
                    if nw > 1:
                        for w in list(si.on_wait)[:-1]:
                            nop_ctr[0] += 1
                            nop = mybir.InstNoOp(
                                name=f"I-waitnop-{nop_ctr[0]}", ins=[],
                                outs=[])
                            nop.engine = inst.engine
                            nop.sync_info = bass_rust.SyncInfo(
                                on_wait=[w], on_update=[])
                            out.append(nop)
                        inst.sync_info = bass_rust.SyncInfo(
                            on_wait=list(si.on_wait)[-1:],
                            on_update=list(si.on_update))
                    elif nw == 0:
                        pending_free_ldw = inst
                    out.append(inst)
                    continue
                si = inst.sync_info
                nw = len(si.on_wait) if si is not None else 0
                if tn in exempt or nw <= 1:
                    out.append(inst)
                    if tn == "InstMatmult":
                        pending_free_ldw = None
                    continue
                waits = list(si.on_wait)
                moved, kept = waits[:-1], waits[-1:]
                if tn == "InstMatmult" and pending_free_ldw is not None \
                        and len(moved) == 1:
                    c = pending_free_ldw
                    csi = c.sync_info
                    c.sync_info = bass_rust.SyncInfo(
                        on_wait=moved,
                        on_update=list(csi.on_update) if csi else [])
                else:
                    for w in moved:
                        nop_ctr[0] += 1
                        nop = mybir.InstNoOp(
                            name=f"I-waitnop-{nop_ctr[0]}", ins=[], outs=[])
                        nop.engine = inst.engine
                        nop.sync_info = bass_rust.SyncInfo(
                            on_wait=[w], on_update=[])
                        out.append(nop)
                inst.sync_info = bass_rust.SyncInfo(
                    on_wait=kept, on_update=list(si.on_update))
                out.append(inst)
                if tn == "InstMatmult":
                    pending_free_ldw = None
            if len(out) != len(insts):
                blk.instructions = out


_NC_CACHE = None


def _get_nc():
    global _NC_CACHE
    if _NC_CACHE is None:
        _NC_CACHE = build_bass()
    return _NC_CACHE


def make_in_maps(hand_landmarks, W1, b1, W2, b2, np_dt=np.float16):
    tw, w2r = fold_weights(W1, b1, W2, b2)
    w2r = w2r.astype(np_dt)
    twd = np.zeros((XROWS, M1), np_dt)
    twd[0:K1] = tw.astype(np_dt)
    twd[64:64 + K1] = twd[0:K1]
    x = np.asarray(hand_landmarks, np.float32).reshape(G, NNODE * CIN)
    xt = np.empty((XROWS, G), np_dt)
    xt[: NNODE * CIN] = x.T
    xt[K1 - 1] = 1.0
    xt[64:64 + K1] = xt[0:K1]
    return [
        {
            "xt": np.ascontiguousarray(xt[:, i * G_CORE:(i + 1) * G_CORE]),
            "tw": twd,
            "w2r": w2r,
        }
        for i in range(N_CORES)
    ]


def gather_out(results):
    full = np.concatenate([results[i]["out"] for i in range(N_CORES)], axis=1)
    return np.ascontiguousarray(full.T).astype(np.float32).reshape(B, S, D2)


def run(in_maps, trace=False, **kw):
    res = bass_utils.run_bass_kernel_spmd(
        _get_nc(), in_maps, core_ids=list(range(N_CORES)), trace=trace, **kw)
    return res


def kernel(hand_landmarks, W1, b1, W2, b2):
    in_maps = make_in_maps(hand_landmarks, W1, b1, W2, b2)
    res = run(in_maps)
    return gather_out(res.results)


# revision 12
# speedup vs baseline: 1.2948x; 1.2948x over previous
"""Trainium2 Bass kernel for nn_HandGNNEncoder (2-layer GCN on 21-node hand
graphs + mean pool), data-parallel over 8 NeuronCores.

Math restructure (exact):
  reference: h1 = relu(A @ (x @ W1) + b1); out = mean_t(A @ (h1 @ W2) + b2)
  mean-pool is linear, so with m[s] = column-mean of A (all > 0):
      out[g] = sum_s m[s] * h1[g,s,:] @ W2 + b2
  m[s] > 0 folds inside the relu:  m*relu(z) = relu(m*z).
  Stage 1 (PE): z[(s,f), g] = TW.T @ x'[g]   with TW[(s',c),(s,f)] =
      m[s]*A[s,s']*W1[c,f], bias row via a constant-1 input row, plus one
      extra column that relu's to the constant 1 (carries b2 in stage 2).
  Stage 2 (PE): out[d, g] = sum_k W2R_k.T @ relu_k  accumulated in PSUM.

v3 performance structure (vs the chunk-major baseline):
  * PE warmup burst: ~20 dependency-free dummy matmuls issued while the
    input DMAs land. The PE HAM clock gate only un-throttles (1.2 GHz ->
    2.4 GHz) after a ~3.4us window of gap-free matmul activity, which the
    synced main loop never provides; the trace showed the whole baseline
    ran at 1.2 GHz.
  * Stage-1 row packing: K=43 uses only rows 0..42 of the 128x128 array.
    xt and TW are duplicated at partition offset 64 so chunk A streams on
    rows 0..42 while chunk B streams concurrently on rows 64..106
    (tile_position (0,0) / (64,0)) - stage-1 wall time halves.
  * k-inner over chunk pairs with [128,1024] PSUM tiles (A|B): one relu
    instruction covers both chunks (the per-instruction overhead on
    ACT/DVE is ~170-230 cycles, so FD=1024 is ~30% cheaper than 2x512),
    PSUM = 3x2 (stage-1) + 1x2 (out accum) = exactly 8 banks.
  * Input DMAs issued from 4 different engine queues in parallel; f16
    output halves the store traffic.
"""

import numpy as np

import concourse.bass as bass
import concourse.mybir as mybir
import concourse.tile as tile
from concourse import bass_utils

# ---- hardcoded problem constants ----
B, S, NNODE, CIN = 64, 512, 21, 2
D1, D2 = 64, 128
G = B * S                      # 32768 graphs
N_CORES = 8
G_CORE = G // N_CORES          # 4096 graphs per core
CHUNK = 512                    # graphs per matmul moving dim
PAIR = 2 * CHUNK               # graphs per pipeline pair
N_PAIRS = G_CORE // PAIR       # 4
K1 = NNODE * CIN + 1           # 43 contraction rows (42 feats + ones row)
KT = 11                        # 1408 / 128 k-tiles for stage 2
M1 = KT * 128                  # 1408 = 1344 (s,f) cols + 1 bias col + 63 pad
XROWS = 107                    # 43 rows + dup at partitions 64..106
SKEW = 3                       # stage-2 of k issued in period k+SKEW
N_WARM = 20                    # HAM warmup matmuls

EDGES = np.array(
    [[0, 1], [1, 2], [2, 3], [3, 4], [0, 5], [5, 6], [6, 7], [7, 8],
     [0, 9], [9, 10], [10, 11], [11, 12], [0, 13], [13, 14], [14, 15],
     [15, 16], [0, 17], [17, 18], [18, 19], [19, 20], [5, 9], [9, 13],
     [13, 17]], dtype=np.int64)


def fold_weights(W1, b1, W2, b2):
    """Fold adjacency, mean-pool and biases into two dense operands."""
    W1 = np.asarray(W1, np.float32)
    b1 = np.asarray(b1, np.float32)
    W2 = np.asarray(W2, np.float32)
    b2 = np.asarray(b2, np.float32)
    A = np.eye(NNODE, dtype=np.float32)
    A[EDGES[:, 1], EDGES[:, 0]] = 1.0
    deg = A.sum(axis=1)
    dis = 1.0 / np.sqrt(deg)
    a_norm = dis[:, None] * A * dis[None, :]          # [t, s] float32
    m = a_norm.mean(axis=0)                           # [21], all > 0

    # tw[(s',c), (s,f)] = m[s] * a_norm[s, s'] * W1[c, f]
    tw = np.zeros((K1, M1), np.float32)
    blk = np.einsum("s,st,cf->tcsf", m, a_norm, W1)   # [s'=t, c, s, f]
    tw[: NNODE * CIN, : NNODE * D1] = blk.reshape(NNODE * CIN, NNODE * D1)
    tw[K1 - 1, : NNODE * D1] = (m[:, None] * b1[None, :]).reshape(-1)
    tw[K1 - 1, NNODE * D1] = 1.0                      # relu's to constant 1

    w2full = np.zeros((M1, D2), np.float32)
    w2full[: NNODE * D1] = np.tile(W2, (NNODE, 1))
    w2full[NNODE * D1] = b2                           # rides the const-1 row
    # device tile is [128, KT*128] with pass-k slice [:, k*128:(k+1)*128]
    w2r = np.ascontiguousarray(
        w2full.reshape(KT, 128, D2).transpose(1, 0, 2).reshape(128, KT * D2))
    return tw, w2r


def build_bass(op_dt="float16"):
    f16 = getattr(mybir.dt, op_dt)
    f32 = mybir.dt.float32
    nc = bass.Bass("TRN2", target_bir_lowering=False, debug=False)
    xt_d = nc.dram_tensor("xt", [XROWS, G_CORE], f16, kind="ExternalInput").ap()
    tw_d = nc.dram_tensor("tw", [XROWS, M1], f16, kind="ExternalInput").ap()
    w2r_d = nc.dram_tensor("w2r", [128, KT * 128], f16,
                           kind="ExternalInput").ap()
    out_d = nc.dram_tensor("out", [D2, G_CORE], f16, kind="ExternalOutput").ap()

    relu = mybir.ActivationFunctionType.Relu
    copyf = mybir.ActivationFunctionType.Copy

    with tile.TileContext(nc) as tc:
        with (
            tc.tile_pool(name="w", bufs=1) as wpool,
            tc.tile_pool(name="rt", bufs=SKEW + 1) as rtpool,
            tc.tile_pool(name="ot", bufs=2) as otpool,
            tc.tile_pool(name="px", bufs=3, space="PSUM") as xpool,
            tc.tile_pool(name="po", bufs=1, space="PSUM") as opool,
        ):
            # --- tiles ---
            tw_t = wpool.tile([XROWS, M1], f16, tag="tw")
            xt_t = wpool.tile([XROWS, G_CORE], f16, tag="xt")
            w2r_t = wpool.tile([128, KT * 128], f16, tag="w2r")
            warm_t = wpool.tile([128, 640], f16, tag="warm")

            # --- input DMAs: each dma_start rides its own HWDGE queue
            # (~27 GiB/s each), so many medium transfers beat few big
            # ones. xt is sliced by pair columns so pair 0's matmuls
            # unblock after two transfers; later pairs land while earlier
            # ones compute. Issue order = first-use order per sequencer. ---
            nc.vector.memset(warm_t, 0.0)
            nc.sync.dma_start(out=tw_t[0:22], in_=tw_d[0:22])
            nc.scalar.dma_start(out=tw_t[22:43], in_=tw_d[22:43])
            nc.sync.dma_start(out=tw_t[64:86], in_=tw_d[64:86])
            nc.scalar.dma_start(out=tw_t[86:107], in_=tw_d[86:107])
            for p in range(N_PAIRS):
                cs = p * PAIR
                nc.scalar.dma_start(out=xt_t[0:43, cs:cs + PAIR],
                                    in_=xt_d[0:43, cs:cs + PAIR])
                nc.sync.dma_start(out=xt_t[64:107, cs:cs + PAIR],
                                  in_=xt_d[64:107, cs:cs + PAIR])
                if p == 1:
                    nc.scalar.dma_start(out=w2r_t[0:32], in_=w2r_d[0:32])
                    nc.sync.dma_start(out=w2r_t[32:64], in_=w2r_d[32:64])
                    nc.scalar.dma_start(out=w2r_t[64:96], in_=w2r_d[64:96])
                    nc.sync.dma_start(out=w2r_t[96:128], in_=w2r_d[96:128])

            # --- HAM warmup: dependency-free matmuls reusing one weight
            # load (ldweights=False on the repeats) -> gap-free PE stream
            # that un-throttles the clock gate (1.2 -> 2.4 GHz) before the
            # real pipeline starts. Writes land in a stage-1 PSUM buffer
            # and are cleared by the first real start=True matmul there. ---
            out_ps = opool.tile([D2, PAIR], f32, tag="po")
            warm_ps = xpool.tile([128, PAIR], f32, tag="px")
            for i in range(N_WARM):
                mm = nc.tensor.matmul(
                    warm_ps[:, 0:CHUNK],
                    lhsT=warm_t[:, 0:128],
                    rhs=warm_t[:, 128:128 + CHUNK],
                    start=True, stop=True,
                    skip_group_check=True,
                )
                if i > 0:
                    mm.ldweights = False

            tw_lo = [tw_t[0:43, k * 128:(k + 1) * 128] for k in range(KT)]
            tw_hi = [tw_t[64:107, k * 128:(k + 1) * 128] for k in range(KT)]
            w2r_sb = [w2r_t[:, k * 128:(k + 1) * 128] for k in range(KT)]

            ACT_KS = {0, 2, 4, 6, 8, 9, 10}   # relu k's on ScalarE; rest DVE

            for p in range(N_PAIRS):
                cs = p * PAIR
                rts = {}

                def mm3(k):
                    nc.tensor.matmul(
                        out_ps[:, 0:CHUNK],
                        lhsT=w2r_sb[k], rhs=rts[k][:, 0:CHUNK],
                        start=(k == 0), stop=(k == KT - 1),
                        skip_group_check=True,
                    )
                    nc.tensor.matmul(
                        out_ps[:, CHUNK:PAIR],
                        lhsT=w2r_sb[k], rhs=rts.pop(k)[:, CHUNK:PAIR],
                        start=(k == 0), stop=(k == KT - 1),
                        skip_group_check=True,
                    )

                for k in range(KT):
                    xk = xpool.tile([128, PAIR], f32, tag="px")
                    nc.tensor.matmul(
                        xk[:, 0:CHUNK],
                        lhsT=tw_lo[k], rhs=xt_t[0:43, cs:cs + CHUNK],
                        start=True, stop=True,
                        tile_position=(0, 0),
                        skip_group_check=True,
                    )
                    nc.tensor.matmul(
                        xk[:, CHUNK:PAIR],
                        lhsT=tw_hi[k],
                        rhs=xt_t[64:107, cs + CHUNK:cs + PAIR],
                        start=True, stop=True,
                        tile_position=(64, 0),
                        skip_group_check=True,
                    )
                    rt = rtpool.tile([128, PAIR], f16, tag="rt")
                    if k in ACT_KS:
                        nc.scalar.activation(out=rt, in_=xk, func=relu)
                    else:
                        nc.vector.tensor_scalar_max(out=rt, in0=xk,
                                                    scalar1=0.0)
                    rts[k] = rt
                    if k >= SKEW:
                        mm3(k - SKEW)
                for k in range(KT - SKEW, KT):
                    mm3(k)

                ot = otpool.tile([D2, PAIR], f16, tag="ot")
                nc.vector.tensor_copy(out=ot, in_=out_ps)
                nc.sync.dma_start(out=out_d[:, cs:cs + PAIR], in_=ot)
    _rebalance_matmul_waits(nc)
    return nc


def _rebalance_matmul_waits(nc):
    """Walrus' TPB ISA structs accept only one sync-wait per instruction on
    the compute engines, but Tile can attach several (PE completion-order +
    cross-engine WAR + DMA). Keep one wait on the instruction and move the
    excess onto the immediately-preceding Ldweights (for matmuls) or onto
    freshly inserted same-engine NoOps - those execute just before on the
    same in-order queue, so waiting there is the same or stronger ordering."""
    import bass_rust
    import concourse.mybir as mybir

    exempt = {"InstEventSemaphore", "InstUnconditionalBranch",
              "InstCall", "InstISA", "InstNoOp"}
    nop_ctr = [0]
    for fn in nc.m.functions:
        for blk in fn.blocks:
            insts = list(blk.instructions)
            out = []
            pending_free_ldw = None
            for inst in insts:
                tn = type(inst).__name__
                if tn == "InstLdweights":
                    si = inst.sync_info
                    nw = len(si.on_wait) if si is not None else 0
                    if nw > 1:
                        for w in list(si.on_wait)[:-1]:
                            nop_ctr[0] += 1
                            nop = mybir.InstNoOp(
                                name=f"I-waitnop-{nop_ctr[0]}", ins=[],
                                outs=[])
                            nop.engine = inst.engine
                            nop.sync_info = bass_rust.SyncInfo(
                                on_wait=[w], on_update=[])
                            out.append(nop)
                        inst.sync_info = bass_rust.SyncInfo(
                            on_wait=list(si.on_wait)[-1:],
                            on_update=list(si.on_update))
                    elif nw == 0:
                        pending_free_ldw = inst
                    out.append(inst)
                    continue
                si = inst.sync_info
                nw = len(si.on_wait) if si is not None else 0
                if tn in exempt or nw <= 1:
                    out.append(inst)
                    if tn == "InstMatmult":
                        pending_free_ldw = None
                    continue
                waits = list(si.on_wait)
                moved, kept = waits[:-1], waits[-1:]
                if tn == "InstMatmult" and pending_free_ldw is not None \
                        and len(moved) == 1:
                    c = pending_free_ldw
                    csi = c.sync_info
                    c.sync_info = bass_rust.SyncInfo(
                        on_wait=moved,
                        on_update=list(csi.on_update) if csi else [])
                else:
                    for w in moved:
                        nop_ctr[0] += 1
                        nop = mybir.InstNoOp(
                            name=f"I-waitnop-{nop_ctr[0]}", ins=[], outs=[])
                        nop.engine = inst.engine
                        nop.sync_info = bass_rust.SyncInfo(
                            on_wait=[w], on_update=[])
                        out.append(nop)
                inst.sync_info = bass_rust.SyncInfo(
                    on_wait=kept, on_update=list(si.on_update))
                out.append(inst)
                if tn == "InstMatmult":
                    pending_free_ldw = None
            if len(out) != len(insts):
                blk.instructions = out


_NC_CACHE = None


def _get_nc():
    global _NC_CACHE
    if _NC_CACHE is None:
        _NC_CACHE = build_bass()
    return _NC_CACHE


def make_in_maps(hand_landmarks, W1, b1, W2, b2, np_dt=np.float16):
    tw, w2r = fold_weights(W1, b1, W2, b2)
    w2r = w2r.astype(np_dt)
    twd = np.zeros((XROWS, M1), np_dt)
    twd[0:K1] = tw.astype(np_dt)
    twd[64:64 + K1] = twd[0:K1]
    x = np.asarray(hand_landmarks, np.float32).reshape(G, NNODE * CIN)
    xt = np.empty((XROWS, G), np_dt)
    xt[: NNODE * CIN] = x.T
    xt[K1 - 1] = 1.0
    xt[64:64 + K1] = xt[0:K1]
    return [
        {
            "xt": np.ascontiguousarray(xt[:, i * G_CORE:(i + 1) * G_CORE]),
            "tw": twd,
            "w2r": w2r,
        }
        for i in range(N_CORES)
    ]


def gather_out(results):
    full = np.concatenate([results[i]["out"] for i in range(N_CORES)], axis=1)
    return np.ascontiguousarray(full.T).astype(np.float32).reshape(B, S, D2)


def run(in_maps, trace=False, **kw):
    res = bass_utils.run_bass_kernel_spmd(
        _get_nc(), in_maps, core_ids=list(range(N_CORES)), trace=trace, **kw)
    return res


def kernel(hand_landmarks, W1, b1, W2, b2):
    in_maps = make_in_maps(hand_landmarks, W1, b1, W2, b2)
    res = run(in_maps)
    return gather_out(res.results)
